# revision 1
# baseline (speedup 1.0000x reference)
"""Trainium2 Bass kernel: Ernie4.5 VisionAttention (varlen attention, 4x512
segments, 16 heads x 80 dim, embed 1280).

Sharding: 8 cores = 2 segment-groups (2x512 tokens each) x 4 head-groups
(4 heads each). Tensor-parallel over heads (qkv column-shard, proj row-shard),
data-parallel over segment pairs. No collectives: per-core proj partials are
summed on the host.

Compute dtype: bf16 operands, fp32 PSUM accumulation.
"""

import sys

if "/opt/trn_rl_repo" not in sys.path:
    sys.path.insert(0, "/opt/trn_rl_repo")

import numpy as np
import ml_dtypes

BF = ml_dtypes.bfloat16

EMBED = 1280
HEADS = 16
HD = 80          # head dim
RH = 40          # rotary half
SEQ = 2048
SEGLEN = 512
N_CORES = 8
HPC = 4          # heads per core
TOK = 1024       # tokens per core (2 segments)
NSEG = 2
NUNITS = 2 * HPC # q units 0..3, k units 4..7
VW = HD + 17     # 97: v block width per head (80 dims + 16 pad + denom col)
VTOT = HPC * VW  # 388
SCALE = HD ** -0.5
KCH = EMBED // 128  # 10

_CACHE = {}


def _build_program():
    import concourse.tile as tile
    from concourse import bacc, mybir

    f32 = mybir.dt.float32
    bf16 = mybir.dt.bfloat16
    AF = mybir.ActivationFunctionType
    ALU = mybir.AluOpType

    nc = bacc.Bacc("TRN2", target_bir_lowering=False, debug=False,
                   num_devices=N_CORES)

    xt_d = nc.dram_tensor("xt", [EMBED + 1, TOK], bf16, kind="ExternalInput").ap()
    wqk_d = nc.dram_tensor("wqk", [EMBED, NUNITS * HD], bf16, kind="ExternalInput").ap()
    wv_d = nc.dram_tensor("wv", [EMBED + 1, VTOT], bf16, kind="ExternalInput").ap()
    wp_d = nc.dram_tensor("wp", [HPC * HD, EMBED], bf16, kind="ExternalInput").ap()
    bias_d = nc.dram_tensor("biasqk", [HD, NUNITS], f32, kind="ExternalInput").ap()
    cos_d = nc.dram_tensor("cosm", [HD, TOK], bf16, kind="ExternalInput").ap()
    sin_d = nc.dram_tensor("sinm", [HD, TOK], bf16, kind="ExternalInput").ap()
    pit_d = nc.dram_tensor("pit", [HD, HD], bf16, kind="ExternalInput").ap()
    out_d = nc.dram_tensor("outT", [EMBED, TOK], f32, kind="ExternalOutput").ap()

    with tile.TileContext(nc) as tc:
        with tc.tile_pool(name="persist", bufs=1) as P:
            # ---- persistent SBUF loads ----
            xt_sb = []
            for e in range(KCH):
                t = P.tile([128, TOK], bf16, name=f"xt{e}", tag=f"xt{e}")
                nc.sync.dma_start(t[:], xt_d[128 * e:128 * (e + 1), :])
                xt_sb.append(t)
            xlast = P.tile([1, TOK], bf16, name="xlast", tag="xlast")
            nc.sync.dma_start(xlast[:], xt_d[EMBED:EMBED + 1, :])

            wqk_sb = []
            for e in range(KCH):
                t = P.tile([128, NUNITS * HD], bf16, name=f"wqk{e}", tag=f"wqk{e}")
                nc.sync.dma_start(t[:], wqk_d[128 * e:128 * (e + 1), :])
                wqk_sb.append(t)

            wv_sb = []
            for e in range(KCH):
                t = P.tile([128, VTOT], bf16, name=f"wv{e}", tag=f"wv{e}")
                nc.sync.dma_start(t[:], wv_d[128 * e:128 * (e + 1), :])
                wv_sb.append(t)
            wvlast = P.tile([1, VTOT], bf16, name="wvlast", tag="wvlast")
            nc.sync.dma_start(wvlast[:], wv_d[EMBED:EMBED + 1, :])

            wp_sb = []
            for j in range(HPC):
                t = P.tile([HD, EMBED], bf16, name=f"wp{j}", tag=f"wp{j}")
                nc.sync.dma_start(t[:], wp_d[HD * j:HD * (j + 1), :])
                wp_sb.append(t)

            bias_sb = P.tile([HD, NUNITS], f32, name="biasqk_sb", tag="biasqk")
            nc.sync.dma_start(bias_sb[:], bias_d[:])
            cos_sb = P.tile([HD, TOK], bf16, name="cos_sb", tag="cosm")
            nc.sync.dma_start(cos_sb[:], cos_d[:])
            sin_sb = P.tile([HD, TOK], bf16, name="sin_sb", tag="sinm")
            nc.sync.dma_start(sin_sb[:], sin_d[:])
            pit_sb = P.tile([HD, HD], bf16, name="pit_sb", tag="pit")
            nc.sync.dma_start(pit_sb[:], pit_d[:])

            # persistent intermediates
            qrot = [P.tile([HD, TOK], bf16, name=f"qrot{u}", tag=f"qrot{u}")
                    for u in range(NUNITS)]
            v_sb = [P.tile([128, VTOT], bf16, name=f"vsb{m}", tag=f"vsb{m}")
                    for m in range(TOK // 128)]
            ctxn = [[P.tile([HD, SEGLEN], bf16, name=f"ctxn{j}_{s}", tag=f"ctxn{j}_{s}")
                     for s in range(NSEG)] for j in range(HPC)]

            # ---- phase B: qk projection + rotary; phase C: v projection ----
            with tc.tile_pool(name="ps_qkv", bufs=2, space="PSUM") as PSQ, \
                 tc.tile_pool(name="ps_swap", bufs=2, space="PSUM") as PSW, \
                 tc.tile_pool(name="ps_v", bufs=2, space="PSUM") as PSV, \
                 tc.tile_pool(name="work", bufs=3) as W:
                for u in range(NUNITS):
                    for s in range(NSEG):
                        sc = slice(SEGLEN * s, SEGLEN * (s + 1))
                        qk_ps = PSQ.tile([HD, SEGLEN], f32, name=f"qkps{u}_{s}", tag="qkps")
                        for e in range(KCH):
                            nc.tensor.matmul(qk_ps[:],
                                             wqk_sb[e][:, HD * u:HD * (u + 1)],
                                             xt_sb[e][:, sc],
                                             start=(e == 0), stop=(e == KCH - 1))
                        q_sb = W.tile([HD, SEGLEN], bf16, name=f"qsb{u}_{s}", tag="qsb")
                        nc.scalar.activation(q_sb[:], qk_ps[:], AF.Identity,
                                             bias=bias_sb[:, u:u + 1])
                        qsw_ps = PSW.tile([HD, SEGLEN], f32, name=f"qsw{u}_{s}", tag="qsw")
                        nc.tensor.matmul(qsw_ps[:], pit_sb[:], q_sb[:],
                                         start=True, stop=True)
                        t1 = W.tile([HD, SEGLEN], bf16, name=f"t1_{u}_{s}", tag="t1")
                        nc.vector.tensor_tensor(t1[:], q_sb[:], cos_sb[:, sc], ALU.mult)
                        t2 = W.tile([HD, SEGLEN], bf16, name=f"t2_{u}_{s}", tag="t2")
                        nc.vector.tensor_tensor(t2[:], qsw_ps[:], sin_sb[:, sc], ALU.mult)
                        nc.vector.tensor_tensor(qrot[u][:, sc], t1[:], t2[:], ALU.add)

                for m in range(TOK // 128):
                    mc = slice(128 * m, 128 * (m + 1))
                    v_ps = PSV.tile([128, VTOT], f32, name=f"vps{m}", tag="vps")
                    for e in range(KCH):
                        nc.tensor.matmul(v_ps[:], xt_sb[e][:, mc], wv_sb[e][:],
                                         start=(e == 0), stop=False)
                    nc.tensor.matmul(v_ps[:], xlast[:, mc], wvlast[:],
                                     start=False, stop=True)
                    nc.scalar.activation(v_sb[m][:], v_ps[:], AF.Copy)

            # ---- phase D: attention per (head j, segment s) ----
            with tc.tile_pool(name="ps_st", bufs=4, space="PSUM") as PST, \
                 tc.tile_pool(name="ps_ctx", bufs=2, space="PSUM") as PSC, \
                 tc.tile_pool(name="workd", bufs=6) as WD:
                for j in range(HPC):
                    for s in range(NSEG):
                        sc = slice(SEGLEN * s, SEGLEN * (s + 1))
                        est = []
                        for tkc in range(SEGLEN // 128):
                            kc = slice(SEGLEN * s + 128 * tkc,
                                       SEGLEN * s + 128 * (tkc + 1))
                            st_ps = PST.tile([128, SEGLEN], f32,
                                             name=f"st{j}_{s}_{tkc}", tag="stps")
                            nc.tensor.matmul(st_ps[:], qrot[HPC + j][:, kc],
                                             qrot[j][:, sc], start=True, stop=True)
                            e_sb = WD.tile([128, SEGLEN], bf16,
                                           name=f"est{j}_{s}_{tkc}", tag="est",
                                           bufs=8)
                            nc.scalar.activation(e_sb[:], st_ps[:], AF.Exp)
                            est.append(e_sb)
                        ctx_ps = PSC.tile([VW, SEGLEN], f32,
                                          name=f"ctxps{j}_{s}", tag="ctxps")
                        for tkc in range(SEGLEN // 128):
                            nc.tensor.matmul(ctx_ps[:],
                                             v_sb[4 * s + tkc][:, VW * j:VW * (j + 1)],
                                             est[tkc][:],
                                             start=(tkc == 0), stop=(tkc == 3))
                        rec = WD.tile([1, SEGLEN], f32, name=f"rec{j}_{s}", tag="rec")
                        nc.vector.reciprocal(rec[:], ctx_ps[VW - 1:VW, :])
                        bc = WD.tile([HD, SEGLEN], f32, name=f"bc{j}_{s}", tag="bc")
                        nc.gpsimd.partition_broadcast(bc[:], rec[:])
                        nc.vector.tensor_tensor(ctxn[j][s][:], ctx_ps[0:HD, :],
                                                bc[:], ALU.mult)

            # ---- phase E: output projection (partial sums over local heads) ----
            with tc.tile_pool(name="ps_o", bufs=2, space="PSUM") as PSO, \
                 tc.tile_pool(name="worke", bufs=3) as WE:
                for e in range(KCH):
                    for s in range(NSEG):
                        o_ps = PSO.tile([128, SEGLEN], f32, name=f"ops{e}_{s}", tag="ops")
                        for j in range(HPC):
                            nc.tensor.matmul(o_ps[:],
                                             wp_sb[j][:, 128 * e:128 * (e + 1)],
                                             ctxn[j][s][:],
                                             start=(j == 0), stop=(j == HPC - 1))
                        o_sb = WE.tile([128, SEGLEN], f32, name=f"osb{e}_{s}", tag="osb")
                        nc.scalar.activation(o_sb[:], o_ps[:], AF.Copy)
                        nc.sync.dma_start(
                            out_d[128 * e:128 * (e + 1),
                                  SEGLEN * s:SEGLEN * (s + 1)], o_sb[:])

    nc.compile()
    return nc


def _prep_inputs(x, rotary_pos_emb, qkv_w, qkv_b):
    """Build per-core input shards (host-side layout/constant prep)."""
    x2 = np.asarray(x, np.float32).reshape(SEQ, EMBED)
    rope = np.asarray(rotary_pos_emb, np.float32)
    qkv_w = np.asarray(qkv_w, np.float32)
    qkv_b = np.asarray(qkv_b, np.float32)

    # rotary multipliers, shared across heads: row d -> r = d % 40
    r_idx = np.arange(HD) % RH
    cos_full = np.cos(rope)[:, r_idx].T.astype(BF)   # [80, 2048]
    sin_full = np.sin(rope)[:, r_idx].T.astype(BF)

    # swap permutation (sign folded): Pi[i, i+40] = -1 (i<40); Pi[i, i-40] = +1
    Pi = np.zeros((HD, HD), np.float32)
    for i in range(RH):
        Pi[i, i + RH] = -1.0
        Pi[i + RH, i] = 1.0
    pit = np.ascontiguousarray(Pi.T).astype(BF)

    in_maps = []
    for c in range(N_CORES):
        sg, hg = divmod(c, HPC)
        toks = slice(TOK * sg, TOK * (sg + 1))
        heads = [HPC * hg + j for j in range(HPC)]

        xa = np.empty((EMBED + 1, TOK), np.float32)
        xa[:EMBED] = x2[toks].T
        xa[EMBED] = 1.0

        wqk = np.empty((EMBED, NUNITS * HD), np.float32)
        bias = np.empty((HD, NUNITS), np.float32)
        for j, h in enumerate(heads):
            wqk[:, HD * j:HD * (j + 1)] = qkv_w[HD * h:HD * (h + 1), :].T * SCALE
            bias[:, j] = qkv_b[HD * h:HD * (h + 1)] * SCALE
            ko = EMBED + HD * h
            wqk[:, HD * (HPC + j):HD * (HPC + j + 1)] = qkv_w[ko:ko + HD, :].T
            bias[:, HPC + j] = qkv_b[ko:ko + HD]

        wv = np.zeros((EMBED + 1, VTOT), np.float32)
        for j, h in enumerate(heads):
            vo = 2 * EMBED + HD * h
            wv[:EMBED, VW * j:VW * j + HD] = qkv_w[vo:vo + HD, :].T
            wv[EMBED, VW * j:VW * j + HD] = qkv_b[vo:vo + HD]
            wv[EMBED, VW * j + VW - 1] = 1.0

        wp = np.empty((HPC * HD, EMBED), np.float32)
        for j, h in enumerate(heads):
            wp[HD * j:HD * (j + 1), :] = _PROJ_W[:, HD * h:HD * (h + 1)].T

        in_maps.append({
            "xt": xa.astype(BF),
            "wqk": wqk.astype(BF),
            "wv": wv.astype(BF),
            "wp": wp.astype(BF),
            "biasqk": bias,
            "cosm": np.ascontiguousarray(cos_full[:, toks]),
            "sinm": np.ascontiguousarray(sin_full[:, toks]),
            "pit": pit,
        })
    return in_maps


_PROJ_W = None


def run_on_device(inputs, trace=False, trace_cores=None):
    """Shard, run on 8 NeuronCores, gather. Returns (output, BassKernelResults)."""
    global _PROJ_W
    from concourse import bass_utils

    x = np.asarray(inputs["x"], np.float32)
    cu = np.asarray(inputs["cu_seqlens"]).tolist()
    assert cu == [0, 512, 1024, 1536, 2048], (
        f"kernel compiled for 4x512 segments, got cu_seqlens={cu}")
    assert x.shape == (SEQ, 1, EMBED)

    _PROJ_W = np.asarray(inputs["proj_w"], np.float32)
    in_maps = _prep_inputs(x, inputs["rotary_pos_emb"],
                           inputs["qkv_w"], inputs["qkv_b"])

    if "nc" not in _CACHE:
        _CACHE["nc"] = _build_program()
    nc = _CACHE["nc"]

    kw = {}
    if trace:
        kw = dict(trace=True, trace_cores=trace_cores or [0])
    res = bass_utils.run_bass_kernel_spmd(nc, in_maps,
                                          core_ids=list(range(N_CORES)), **kw)

    proj_b = np.asarray(inputs["proj_b"], np.float32)
    out = np.empty((SEQ, EMBED), np.float32)
    for sg in range(2):
        acc = res.results[HPC * sg + 0]["outT"].astype(np.float32).copy()
        for hg in range(1, HPC):
            acc += res.results[HPC * sg + hg]["outT"]
        out[TOK * sg:TOK * (sg + 1)] = acc.T
    out += proj_b
    return out.reshape(SEQ, 1, EMBED), res


def kernel(**inputs):
    out, _ = run_on_device(inputs, trace=False)
    return out


# revision 8
# speedup vs baseline: 1.0291x; 1.0291x over previous
"""Trainium2 Bass kernel: Ernie4.5 VisionAttention (varlen attention, 4x512
segments, 16 heads x 80 dim, embed 1280).

Sharding: 8 cores = 2 segment-groups (2x512 tokens each) x 4 head-groups
(4 heads each). Tensor-parallel over heads (qkv column-shard, proj row-shard),
data-parallel over segment pairs. No collectives: per-core proj partials are
summed on the host.

Compute dtype: bf16 operands, fp32 PSUM accumulation.
"""

import sys

if "/opt/trn_rl_repo" not in sys.path:
    sys.path.insert(0, "/opt/trn_rl_repo")

import numpy as np
import ml_dtypes

BF = ml_dtypes.bfloat16

EMBED = 1280
HEADS = 16
HD = 80          # head dim
RH = 40          # rotary half
SEQ = 2048
SEGLEN = 512
N_CORES = 8
HPC = 4          # heads per core
TOK = 1024       # tokens per core (2 segments)
NSEG = 2
NUNITS = 2 * HPC # q units 0..3, k units 4..7
VW = HD + 17     # 97: v block width per head (80 dims + 16 pad + denom col)
VTOT = HPC * VW  # 388
SCALE = HD ** -0.5
KCH = EMBED // 128  # 10

_CACHE = {}


def _build_program():
    import concourse.tile as tile
    from concourse import bacc, mybir

    f32 = mybir.dt.float32
    bf16 = mybir.dt.bfloat16
    AF = mybir.ActivationFunctionType
    ALU = mybir.AluOpType

    nc = bacc.Bacc("TRN2", target_bir_lowering=False, debug=False,
                   num_devices=N_CORES)

    xt_d = nc.dram_tensor("xt", [EMBED + 1, TOK], bf16, kind="ExternalInput").ap()
    wqk_d = nc.dram_tensor("wqk", [EMBED, NUNITS * HD], bf16, kind="ExternalInput").ap()
    wv_d = nc.dram_tensor("wv", [EMBED + 1, VTOT], bf16, kind="ExternalInput").ap()
    wp_d = nc.dram_tensor("wp", [HPC * HD, EMBED], bf16, kind="ExternalInput").ap()
    bias_d = nc.dram_tensor("biasqk", [HD, NUNITS], f32, kind="ExternalInput").ap()
    cos_d = nc.dram_tensor("cosm", [HD, TOK], bf16, kind="ExternalInput").ap()
    sin_d = nc.dram_tensor("sinm", [HD, TOK], bf16, kind="ExternalInput").ap()
    pit_d = nc.dram_tensor("pit", [HD, HD], bf16, kind="ExternalInput").ap()
    out_d = nc.dram_tensor("outT", [EMBED, TOK], f32, kind="ExternalOutput").ap()

    with tile.TileContext(nc) as tc:
        with tc.tile_pool(name="persist", bufs=1) as P:
            # ---- persistent SBUF loads ----
            # interleave xt/wqk chunk loads so the first qk matmuls can start
            # after the first chunks land instead of after all input DMAs
            xt_sb = []
            wqk_sb = []
            for e in range(KCH):
                t = P.tile([128, TOK], bf16, name=f"xt{e}", tag=f"xt{e}")
                nc.sync.dma_start(t[:], xt_d[128 * e:128 * (e + 1), :])
                xt_sb.append(t)
                t = P.tile([128, NUNITS * HD], bf16, name=f"wqk{e}", tag=f"wqk{e}")
                nc.sync.dma_start(t[:], wqk_d[128 * e:128 * (e + 1), :])
                wqk_sb.append(t)

            bias_sb = P.tile([HD, NUNITS], f32, name="biasqk_sb", tag="biasqk")
            nc.sync.dma_start(bias_sb[:], bias_d[:])
            cos_sb = P.tile([HD, TOK], bf16, name="cos_sb", tag="cosm")
            nc.sync.dma_start(cos_sb[:], cos_d[:])
            sin_sb = P.tile([HD, TOK], bf16, name="sin_sb", tag="sinm")
            nc.sync.dma_start(sin_sb[:], sin_d[:])
            pit_sb = P.tile([HD, HD], bf16, name="pit_sb", tag="pit")
            nc.sync.dma_start(pit_sb[:], pit_d[:])

            wv_sb = []
            for e in range(KCH):
                t = P.tile([128, VTOT], bf16, name=f"wv{e}", tag=f"wv{e}")
                nc.sync.dma_start(t[:], wv_d[128 * e:128 * (e + 1), :])
                wv_sb.append(t)
            wvlast = P.tile([1, VTOT], bf16, name="wvlast", tag="wvlast")
            nc.sync.dma_start(wvlast[:], wv_d[EMBED:EMBED + 1, :])
            xlast = P.tile([1, TOK], bf16, name="xlast", tag="xlast")
            nc.sync.dma_start(xlast[:], xt_d[EMBED:EMBED + 1, :])

            wp_sb = []
            for j in range(HPC):
                t = P.tile([HD, EMBED], bf16, name=f"wp{j}", tag=f"wp{j}")
                nc.sync.dma_start(t[:], wp_d[HD * j:HD * (j + 1), :])
                wp_sb.append(t)

            # persistent intermediates
            qrot = [P.tile([HD, TOK], bf16, name=f"qrot{u}", tag=f"qrot{u}")
                    for u in range(NUNITS)]
            v_sb = [P.tile([128, VTOT], bf16, name=f"vsb{m}", tag=f"vsb{m}")
                    for m in range(TOK // 128)]
            ctxn = [[P.tile([HD, SEGLEN], bf16, name=f"ctxn{j}_{s}", tag=f"ctxn{j}_{s}")
                     for s in range(NSEG)] for j in range(HPC)]

            # ---- phase B: qk projection + rotary; phase C: v projection ----
            with tc.tile_pool(name="ps_qkv", bufs=2, space="PSUM") as PSQ, \
                 tc.tile_pool(name="ps_swap", bufs=2, space="PSUM") as PSW, \
                 tc.tile_pool(name="ps_v", bufs=2, space="PSUM") as PSV, \
                 tc.tile_pool(name="work", bufs=3) as W:
                for u in range(NUNITS):
                    for s in range(NSEG):
                        sc = slice(SEGLEN * s, SEGLEN * (s + 1))
                        qk_ps = PSQ.tile([HD, SEGLEN], f32, name=f"qkps{u}_{s}", tag="qkps")
                        for e in range(KCH):
                            nc.tensor.matmul(qk_ps[:],
                                             wqk_sb[e][:, HD * u:HD * (u + 1)],
                                             xt_sb[e][:, sc],
                                             start=(e == 0), stop=(e == KCH - 1))
                        q_sb = W.tile([HD, SEGLEN], bf16, name=f"qsb{u}_{s}", tag="qsb")
                        nc.scalar.activation(q_sb[:], qk_ps[:], AF.Identity,
                                             bias=bias_sb[:, u:u + 1])
                        qsw_ps = PSW.tile([HD, SEGLEN], f32, name=f"qsw{u}_{s}", tag="qsw")
                        nc.tensor.matmul(qsw_ps[:], pit_sb[:], q_sb[:],
                                         start=True, stop=True)
                        # f32 intermediates: only the final add rounds to bf16
                        t1 = W.tile([HD, SEGLEN], f32, name=f"t1_{u}_{s}", tag="t1")
                        nc.vector.tensor_tensor(t1[:], q_sb[:], cos_sb[:, sc], ALU.mult)
                        t2 = W.tile([HD, SEGLEN], f32, name=f"t2_{u}_{s}", tag="t2")
                        nc.vector.tensor_tensor(t2[:], qsw_ps[:], sin_sb[:, sc], ALU.mult)
                        nc.vector.tensor_tensor(qrot[u][:, sc], t1[:], t2[:], ALU.add)

                    # interleave one v chunk per qk unit to keep PE streaming
                    mc = slice(128 * u, 128 * (u + 1))
                    v_ps = PSV.tile([128, VTOT], f32, name=f"vps{u}", tag="vps")
                    for e in range(KCH):
                        nc.tensor.matmul(v_ps[:], xt_sb[e][:, mc], wv_sb[e][:],
                                         start=(e == 0), stop=False)
                    nc.tensor.matmul(v_ps[:], xlast[:, mc], wvlast[:],
                                     start=False, stop=True)
                    nc.vector.tensor_copy(v_sb[u][:], v_ps[:])

            # ---- phase D: attention per (head j, segment s) ----
            with tc.tile_pool(name="ps_st", bufs=6, space="PSUM") as PST, \
                 tc.tile_pool(name="ps_ctx", bufs=2, space="PSUM") as PSC, \
                 tc.tile_pool(name="workd", bufs=6) as WD:
                for j in range(HPC):
                    for s in range(NSEG):
                        sc = slice(SEGLEN * s, SEGLEN * (s + 1))
                        est = []
                        for tkc in range(SEGLEN // 128):
                            kc = slice(SEGLEN * s + 128 * tkc,
                                       SEGLEN * s + 128 * (tkc + 1))
                            st_ps = PST.tile([128, SEGLEN], f32,
                                             name=f"st{j}_{s}_{tkc}", tag="stps")
                            nc.tensor.matmul(st_ps[:], qrot[HPC + j][:, kc],
                                             qrot[j][:, sc], start=True, stop=True)
                            e_sb = WD.tile([128, SEGLEN], bf16,
                                           name=f"est{j}_{s}_{tkc}", tag="est",
                                           bufs=8)
                            nc.scalar.activation(e_sb[:], st_ps[:], AF.Exp)
                            est.append(e_sb)
                        ctx_ps = PSC.tile([VW, SEGLEN], f32,
                                          name=f"ctxps{j}_{s}", tag="ctxps")
                        for tkc in range(SEGLEN // 128):
                            nc.tensor.matmul(ctx_ps[:],
                                             v_sb[4 * s + tkc][:, VW * j:VW * (j + 1)],
                                             est[tkc][:],
                                             start=(tkc == 0), stop=(tkc == 3))
                        rec = WD.tile([1, SEGLEN], f32, name=f"rec{j}_{s}", tag="rec")
                        nc.vector.reciprocal(rec[:], ctx_ps[VW - 1:VW, :])
                        bc = WD.tile([HD, SEGLEN], f32, name=f"bc{j}_{s}", tag="bc")
                        nc.gpsimd.partition_broadcast(bc[:], rec[:])
                        nc.vector.tensor_tensor(ctxn[j][s][:], ctx_ps[0:HD, :],
                                                bc[:], ALU.mult)

            # ---- phase E: output projection (partial sums over local heads) ----
            with tc.tile_pool(name="ps_o", bufs=4, space="PSUM") as PSO, \
                 tc.tile_pool(name="worke", bufs=4) as WE:
                for e in range(KCH):
                    for s in range(NSEG):
                        o_ps = PSO.tile([128, SEGLEN], f32, name=f"ops{e}_{s}", tag="ops")
                        for j in range(HPC):
                            nc.tensor.matmul(o_ps[:],
                                             wp_sb[j][:, 128 * e:128 * (e + 1)],
                                             ctxn[j][s][:],
                                             start=(j == 0), stop=(j == HPC - 1))
                        o_sb = WE.tile([128, SEGLEN], f32, name=f"osb{e}_{s}", tag="osb")
                        nc.scalar.activation(o_sb[:], o_ps[:], AF.Copy)
                        nc.sync.dma_start(
                            out_d[128 * e:128 * (e + 1),
                                  SEGLEN * s:SEGLEN * (s + 1)], o_sb[:])

    nc.compile()
    return nc


def _prep_inputs(x, rotary_pos_emb, qkv_w, qkv_b):
    """Build per-core input shards (host-side layout/constant prep)."""
    x2 = np.asarray(x, np.float32).reshape(SEQ, EMBED)
    rope = np.asarray(rotary_pos_emb, np.float32)
    qkv_w = np.asarray(qkv_w, np.float32)
    qkv_b = np.asarray(qkv_b, np.float32)

    # rotary multipliers, shared across heads: row d -> r = d % 40
    r_idx = np.arange(HD) % RH
    cos_full = np.cos(rope)[:, r_idx].T.astype(BF)   # [80, 2048]
    sin_full = np.sin(rope)[:, r_idx].T.astype(BF)

    # swap permutation (sign folded): Pi[i, i+40] = -1 (i<40); Pi[i, i-40] = +1
    Pi = np.zeros((HD, HD), np.float32)
    for i in range(RH):
        Pi[i, i + RH] = -1.0
        Pi[i + RH, i] = 1.0
    pit = np.ascontiguousarray(Pi.T).astype(BF)

    in_maps = []
    for c in range(N_CORES):
        sg, hg = divmod(c, HPC)
        toks = slice(TOK * sg, TOK * (sg + 1))
        heads = [HPC * hg + j for j in range(HPC)]

        xa = np.empty((EMBED + 1, TOK), np.float32)
        xa[:EMBED] = x2[toks].T
        xa[EMBED] = 1.0

        wqk = np.empty((EMBED, NUNITS * HD), np.float32)
        bias = np.empty((HD, NUNITS), np.float32)
        for j, h in enumerate(heads):
            wqk[:, HD * j:HD * (j + 1)] = qkv_w[HD * h:HD * (h + 1), :].T * SCALE
            bias[:, j] = qkv_b[HD * h:HD * (h + 1)] * SCALE
            ko = EMBED + HD * h
            wqk[:, HD * (HPC + j):HD * (HPC + j + 1)] = qkv_w[ko:ko + HD, :].T
            bias[:, HPC + j] = qkv_b[ko:ko + HD]

        wv = np.zeros((EMBED + 1, VTOT), np.float32)
        for j, h in enumerate(heads):
            vo = 2 * EMBED + HD * h
            wv[:EMBED, VW * j:VW * j + HD] = qkv_w[vo:vo + HD, :].T
            wv[EMBED, VW * j:VW * j + HD] = qkv_b[vo:vo + HD]
            wv[EMBED, VW * j + VW - 1] = 1.0

        wp = np.empty((HPC * HD, EMBED), np.float32)
        for j, h in enumerate(heads):
            wp[HD * j:HD * (j + 1), :] = _PROJ_W[:, HD * h:HD * (h + 1)].T

        in_maps.append({
            "xt": xa.astype(BF),
            "wqk": wqk.astype(BF),
            "wv": wv.astype(BF),
            "wp": wp.astype(BF),
            "biasqk": bias,
            "cosm": np.ascontiguousarray(cos_full[:, toks]),
            "sinm": np.ascontiguousarray(sin_full[:, toks]),
            "pit": pit,
        })
    return in_maps


_PROJ_W = None


def run_on_device(inputs, trace=False, trace_cores=None):
    """Shard, run on 8 NeuronCores, gather. Returns (output, BassKernelResults)."""
    global _PROJ_W
    from concourse import bass_utils

    x = np.asarray(inputs["x"], np.float32)
    cu = np.asarray(inputs["cu_seqlens"]).tolist()
    assert cu == [0, 512, 1024, 1536, 2048], (
        f"kernel compiled for 4x512 segments, got cu_seqlens={cu}")
    assert x.shape == (SEQ, 1, EMBED)

    _PROJ_W = np.asarray(inputs["proj_w"], np.float32)
    in_maps = _prep_inputs(x, inputs["rotary_pos_emb"],
                           inputs["qkv_w"], inputs["qkv_b"])

    if "nc" not in _CACHE:
        _CACHE["nc"] = _build_program()
    nc = _CACHE["nc"]

    kw = {}
    if trace:
        kw = dict(trace=True, trace_cores=trace_cores or [0])
    res = bass_utils.run_bass_kernel_spmd(nc, in_maps,
                                          core_ids=list(range(N_CORES)), **kw)

    proj_b = np.asarray(inputs["proj_b"], np.float32)
    out = np.empty((SEQ, EMBED), np.float32)
    for sg in range(2):
        acc = res.results[HPC * sg + 0]["outT"].astype(np.float32).copy()
        for hg in range(1, HPC):
            acc += res.results[HPC * sg + hg]["outT"]
        out[TOK * sg:TOK * (sg + 1)] = acc.T
    out += proj_b
    return out.reshape(SEQ, 1, EMBED), res


def kernel(**inputs):
    out, _ = run_on_device(inputs, trace=False)
    return out


# revision 9
# speedup vs baseline: 1.2361x; 1.2012x over previous
"""Trainium2 Bass kernel: Ernie4.5 VisionAttention (varlen attention, 4x512
segments, 16 heads x 80 dim, embed 1280).

Sharding: 8 cores = 2 segment-groups (2x512 tokens each) x 4 head-groups
(4 heads each). Tensor-parallel over heads (qkv column-shard, proj row-shard),
data-parallel over segment pairs. No collectives: per-core proj partials are
summed on the host.

Compute dtype: bf16 operands, fp32 PSUM accumulation.
"""

import sys

if "/opt/trn_rl_repo" not in sys.path:
    sys.path.insert(0, "/opt/trn_rl_repo")

import numpy as np
import ml_dtypes

BF = ml_dtypes.bfloat16

EMBED = 1280
HEADS = 16
HD = 80          # head dim
RH = 40          # rotary half
SEQ = 2048
SEGLEN = 512
N_CORES = 8
HPC = 4          # heads per core
TOK = 1024       # tokens per core (2 segments)
NSEG = 2
NUNITS = 2 * HPC # q units 0..3, k units 4..7
VW = HD          # v block width per head (denominator computed separately)
VTOT = HPC * VW  # 320
SCALE = HD ** -0.5
KCH = EMBED // 128  # 10

_CACHE = {}


def _build_program():
    import concourse.tile as tile
    from concourse import bacc, mybir

    f32 = mybir.dt.float32
    bf16 = mybir.dt.bfloat16
    AF = mybir.ActivationFunctionType
    ALU = mybir.AluOpType

    nc = bacc.Bacc("TRN2", target_bir_lowering=False, debug=False,
                   num_devices=N_CORES)

    xt_d = nc.dram_tensor("xt", [EMBED + 1, TOK], bf16, kind="ExternalInput").ap()
    wqk_d = nc.dram_tensor("wqk", [EMBED, NUNITS * HD], bf16, kind="ExternalInput").ap()
    wv_d = nc.dram_tensor("wv", [EMBED + 1, VTOT], bf16, kind="ExternalInput").ap()
    wp_d = nc.dram_tensor("wp", [HPC * HD, EMBED], bf16, kind="ExternalInput").ap()
    bias_d = nc.dram_tensor("biasqk", [HD, NUNITS], f32, kind="ExternalInput").ap()
    cos_d = nc.dram_tensor("cosm", [HD, TOK], bf16, kind="ExternalInput").ap()
    sin_d = nc.dram_tensor("sinm", [HD, TOK], bf16, kind="ExternalInput").ap()
    pit_d = nc.dram_tensor("pit", [HD, HD], bf16, kind="ExternalInput").ap()
    out_d = nc.dram_tensor("outT", [EMBED, TOK], f32, kind="ExternalOutput").ap()

    with tile.TileContext(nc) as tc:
        with tc.tile_pool(name="persist", bufs=1) as P:
            # ---- persistent SBUF loads ----
            # interleave xt/wqk chunk loads so the first qk matmuls can start
            # after the first chunks land instead of after all input DMAs
            xt_sb = []
            wqk_sb = []
            for e in range(KCH):
                t = P.tile([128, TOK], bf16, name=f"xt{e}", tag=f"xt{e}")
                nc.sync.dma_start(t[:], xt_d[128 * e:128 * (e + 1), :])
                xt_sb.append(t)
                t = P.tile([128, NUNITS * HD], bf16, name=f"wqk{e}", tag=f"wqk{e}")
                nc.sync.dma_start(t[:], wqk_d[128 * e:128 * (e + 1), :])
                wqk_sb.append(t)

            bias_sb = P.tile([HD, NUNITS], f32, name="biasqk_sb", tag="biasqk")
            nc.sync.dma_start(bias_sb[:], bias_d[:])
            cos_sb = P.tile([HD, TOK], bf16, name="cos_sb", tag="cosm")
            nc.sync.dma_start(cos_sb[:], cos_d[:])
            sin_sb = P.tile([HD, TOK], bf16, name="sin_sb", tag="sinm")
            nc.sync.dma_start(sin_sb[:], sin_d[:])
            pit_sb = P.tile([HD, HD], bf16, name="pit_sb", tag="pit")
            nc.sync.dma_start(pit_sb[:], pit_d[:])
            ones_sb = P.tile([128, 1], bf16, name="ones_sb", tag="ones")
            nc.gpsimd.memset(ones_sb[:], 1.0)

            wv_sb = []
            for e in range(KCH):
                t = P.tile([128, VTOT], bf16, name=f"wv{e}", tag=f"wv{e}")
                nc.sync.dma_start(t[:], wv_d[128 * e:128 * (e + 1), :])
                wv_sb.append(t)
            wvlast = P.tile([1, VTOT], bf16, name="wvlast", tag="wvlast")
            nc.sync.dma_start(wvlast[:], wv_d[EMBED:EMBED + 1, :])
            xlast = P.tile([1, TOK], bf16, name="xlast", tag="xlast")
            nc.sync.dma_start(xlast[:], xt_d[EMBED:EMBED + 1, :])

            wp_sb = []
            for j in range(HPC):
                t = P.tile([HD, EMBED], bf16, name=f"wp{j}", tag=f"wp{j}")
                nc.sync.dma_start(t[:], wp_d[HD * j:HD * (j + 1), :])
                wp_sb.append(t)

            # persistent intermediates
            qrot = [P.tile([HD, TOK], bf16, name=f"qrot{u}", tag=f"qrot{u}")
                    for u in range(NUNITS)]
            v_sb = [P.tile([128, VTOT], bf16, name=f"vsb{m}", tag=f"vsb{m}")
                    for m in range(TOK // 128)]
            ctxn = [[P.tile([HD, SEGLEN], bf16, name=f"ctxn{j}_{s}", tag=f"ctxn{j}_{s}")
                     for s in range(NSEG)] for j in range(HPC)]

            # ---- phase B: qk projection + rotary; phase C: v projection ----
            with tc.tile_pool(name="ps_qkv", bufs=2, space="PSUM") as PSQ, \
                 tc.tile_pool(name="ps_swap", bufs=2, space="PSUM") as PSW, \
                 tc.tile_pool(name="ps_v", bufs=2, space="PSUM") as PSV, \
                 tc.tile_pool(name="work", bufs=3) as W:
                for u in range(NUNITS):
                    for s in range(NSEG):
                        sc = slice(SEGLEN * s, SEGLEN * (s + 1))
                        qk_ps = PSQ.tile([HD, SEGLEN], f32, name=f"qkps{u}_{s}", tag="qkps")
                        for e in range(KCH):
                            nc.tensor.matmul(qk_ps[:],
                                             wqk_sb[e][:, HD * u:HD * (u + 1)],
                                             xt_sb[e][:, sc],
                                             start=(e == 0), stop=(e == KCH - 1))
                        q_sb = W.tile([HD, SEGLEN], bf16, name=f"qsb{u}_{s}", tag="qsb")
                        nc.scalar.activation(q_sb[:], qk_ps[:], AF.Identity,
                                             bias=bias_sb[:, u:u + 1])
                        qsw_ps = PSW.tile([HD, SEGLEN], f32, name=f"qsw{u}_{s}", tag="qsw")
                        nc.tensor.matmul(qsw_ps[:], pit_sb[:], q_sb[:],
                                         start=True, stop=True)
                        # f32 intermediates: only the final add rounds to bf16
                        t1 = W.tile([HD, SEGLEN], f32, name=f"t1_{u}_{s}", tag="t1")
                        nc.vector.tensor_tensor(t1[:], q_sb[:], cos_sb[:, sc], ALU.mult)
                        t2 = W.tile([HD, SEGLEN], f32, name=f"t2_{u}_{s}", tag="t2")
                        nc.vector.tensor_tensor(t2[:], qsw_ps[:], sin_sb[:, sc], ALU.mult)
                        nc.vector.tensor_tensor(qrot[u][:, sc], t1[:], t2[:], ALU.add)

                    # interleave one v chunk per qk unit to keep PE streaming
                    mc = slice(128 * u, 128 * (u + 1))
                    v_ps = PSV.tile([128, VTOT], f32, name=f"vps{u}", tag="vps")
                    for e in range(KCH):
                        nc.tensor.matmul(v_ps[:], xt_sb[e][:, mc], wv_sb[e][:],
                                         start=(e == 0), stop=False)
                    nc.tensor.matmul(v_ps[:], xlast[:, mc], wvlast[:],
                                     start=False, stop=True)
                    nc.vector.tensor_copy(v_sb[u][:], v_ps[:])

            # ---- phase D: attention per (head j, segment s) ----
            with tc.tile_pool(name="ps_st", bufs=4, space="PSUM") as PST, \
                 tc.tile_pool(name="ps_ctx", bufs=2, space="PSUM") as PSC, \
                 tc.tile_pool(name="ps_den", bufs=2, space="PSUM") as PSD, \
                 tc.tile_pool(name="workd", bufs=6) as WD:
                for j in range(HPC):
                    for s in range(NSEG):
                        sc = slice(SEGLEN * s, SEGLEN * (s + 1))
                        est = []
                        for tkc in range(SEGLEN // 128):
                            kc = slice(SEGLEN * s + 128 * tkc,
                                       SEGLEN * s + 128 * (tkc + 1))
                            st_ps = PST.tile([128, SEGLEN], f32,
                                             name=f"st{j}_{s}_{tkc}", tag="stps")
                            nc.tensor.matmul(st_ps[:], qrot[HPC + j][:, kc],
                                             qrot[j][:, sc], start=True, stop=True)
                            e_sb = WD.tile([128, SEGLEN], bf16,
                                           name=f"est{j}_{s}_{tkc}", tag="est",
                                           bufs=8)
                            nc.scalar.activation(e_sb[:], st_ps[:], AF.Exp)
                            est.append(e_sb)
                        ctx_ps = PSC.tile([VW, SEGLEN], f32,
                                          name=f"ctxps{j}_{s}", tag="ctxps")
                        den_ps = PSD.tile([1, SEGLEN], f32,
                                          name=f"denps{j}_{s}", tag="denps")
                        for tkc in range(SEGLEN // 128):
                            nc.tensor.matmul(ctx_ps[:],
                                             v_sb[4 * s + tkc][:, VW * j:VW * (j + 1)],
                                             est[tkc][:],
                                             start=(tkc == 0), stop=(tkc == 3))
                            nc.tensor.matmul(den_ps[:], ones_sb[:], est[tkc][:],
                                             start=(tkc == 0), stop=(tkc == 3))
                        rec = WD.tile([1, SEGLEN], f32, name=f"rec{j}_{s}", tag="rec")
                        nc.vector.reciprocal_approx_fast(rec[:], den_ps[:])
                        bc = WD.tile([HD, SEGLEN], f32, name=f"bc{j}_{s}", tag="bc")
                        nc.gpsimd.partition_broadcast(bc[:], rec[:])
                        nc.vector.tensor_tensor(ctxn[j][s][:], ctx_ps[:],
                                                bc[:], ALU.mult)

            # ---- phase E: output projection (partial sums over local heads) ----
            with tc.tile_pool(name="ps_o", bufs=4, space="PSUM") as PSO, \
                 tc.tile_pool(name="worke", bufs=4) as WE:
                for e in range(KCH):
                    for s in range(NSEG):
                        o_ps = PSO.tile([128, SEGLEN], f32, name=f"ops{e}_{s}", tag="ops")
                        for j in range(HPC):
                            nc.tensor.matmul(o_ps[:],
                                             wp_sb[j][:, 128 * e:128 * (e + 1)],
                                             ctxn[j][s][:],
                                             start=(j == 0), stop=(j == HPC - 1))
                        o_sb = WE.tile([128, SEGLEN], f32, name=f"osb{e}_{s}", tag="osb")
                        nc.scalar.activation(o_sb[:], o_ps[:], AF.Copy)
                        nc.sync.dma_start(
                            out_d[128 * e:128 * (e + 1),
                                  SEGLEN * s:SEGLEN * (s + 1)], o_sb[:])

    nc.compile()
    return nc


def _prep_inputs(x, rotary_pos_emb, qkv_w, qkv_b):
    """Build per-core input shards (host-side layout/constant prep)."""
    x2 = np.asarray(x, np.float32).reshape(SEQ, EMBED)
    rope = np.asarray(rotary_pos_emb, np.float32)
    qkv_w = np.asarray(qkv_w, np.float32)
    qkv_b = np.asarray(qkv_b, np.float32)

    # rotary multipliers, shared across heads: row d -> r = d % 40
    r_idx = np.arange(HD) % RH
    cos_full = np.cos(rope)[:, r_idx].T.astype(BF)   # [80, 2048]
    sin_full = np.sin(rope)[:, r_idx].T.astype(BF)

    # swap permutation (sign folded): Pi[i, i+40] = -1 (i<40); Pi[i, i-40] = +1
    Pi = np.zeros((HD, HD), np.float32)
    for i in range(RH):
        Pi[i, i + RH] = -1.0
        Pi[i + RH, i] = 1.0
    pit = np.ascontiguousarray(Pi.T).astype(BF)

    in_maps = []
    for c in range(N_CORES):
        sg, hg = divmod(c, HPC)
        toks = slice(TOK * sg, TOK * (sg + 1))
        heads = [HPC * hg + j for j in range(HPC)]

        xa = np.empty((EMBED + 1, TOK), np.float32)
        xa[:EMBED] = x2[toks].T
        xa[EMBED] = 1.0

        wqk = np.empty((EMBED, NUNITS * HD), np.float32)
        bias = np.empty((HD, NUNITS), np.float32)
        for j, h in enumerate(heads):
            wqk[:, HD * j:HD * (j + 1)] = qkv_w[HD * h:HD * (h + 1), :].T * SCALE
            bias[:, j] = qkv_b[HD * h:HD * (h + 1)] * SCALE
            ko = EMBED + HD * h
            wqk[:, HD * (HPC + j):HD * (HPC + j + 1)] = qkv_w[ko:ko + HD, :].T
            bias[:, HPC + j] = qkv_b[ko:ko + HD]

        wv = np.zeros((EMBED + 1, VTOT), np.float32)
        for j, h in enumerate(heads):
            vo = 2 * EMBED + HD * h
            wv[:EMBED, VW * j:VW * j + HD] = qkv_w[vo:vo + HD, :].T
            wv[EMBED, VW * j:VW * j + HD] = qkv_b[vo:vo + HD]

        wp = np.empty((HPC * HD, EMBED), np.float32)
        for j, h in enumerate(heads):
            wp[HD * j:HD * (j + 1), :] = _PROJ_W[:, HD * h:HD * (h + 1)].T

        in_maps.append({
            "xt": xa.astype(BF),
            "wqk": wqk.astype(BF),
            "wv": wv.astype(BF),
            "wp": wp.astype(BF),
            "biasqk": bias,
            "cosm": np.ascontiguousarray(cos_full[:, toks]),
            "sinm": np.ascontiguousarray(sin_full[:, toks]),
            "pit": pit,
        })
    return in_maps


_PROJ_W = None


def run_on_device(inputs, trace=False, trace_cores=None):
    """Shard, run on 8 NeuronCores, gather. Returns (output, BassKernelResults)."""
    global _PROJ_W
    from concourse import bass_utils

    x = np.asarray(inputs["x"], np.float32)
    cu = np.asarray(inputs["cu_seqlens"]).tolist()
    assert cu == [0, 512, 1024, 1536, 2048], (
        f"kernel compiled for 4x512 segments, got cu_seqlens={cu}")
    assert x.shape == (SEQ, 1, EMBED)

    _PROJ_W = np.asarray(inputs["proj_w"], np.float32)
    in_maps = _prep_inputs(x, inputs["rotary_pos_emb"],
                           inputs["qkv_w"], inputs["qkv_b"])

    if "nc" not in _CACHE:
        _CACHE["nc"] = _build_program()
    nc = _CACHE["nc"]

    kw = {}
    if trace:
        kw = dict(trace=True, trace_cores=trace_cores or [0])
    res = bass_utils.run_bass_kernel_spmd(nc, in_maps,
                                          core_ids=list(range(N_CORES)), **kw)

    proj_b = np.asarray(inputs["proj_b"], np.float32)
    out = np.empty((SEQ, EMBED), np.float32)
    for sg in range(2):
        acc = res.results[HPC * sg + 0]["outT"].astype(np.float32).copy()
        for hg in range(1, HPC):
            acc += res.results[HPC * sg + hg]["outT"]
        out[TOK * sg:TOK * (sg + 1)] = acc.T
    out += proj_b
    return out.reshape(SEQ, 1, EMBED), res


def kernel(**inputs):
    out, _ = run_on_device(inputs, trace=False)
    return out
